# revision 1
# baseline (speedup 1.0000x reference)
"""Complex-magnitude MaxPool2d (k=2, s=2) Trainium2 Bass kernel.

Input  x:  [16, 2, 64, 224, 224] f32  (plane 0 = real, plane 1 = imag)
Output:    [16, 2, 64, 112, 112] f32  (value of the window element with the
                                       largest |z|^2 = re^2 + im^2)

Sharding: pure data parallel over batch: 16 / 8 cores = 2 examples per core.
Per core the 2(batch) x 64(channel) = 128 image planes map 1:1 onto the 128
SBUF partitions; compute runs on 14-row subchunks (the first 14 rows are
split 4+10 so the first square/DVE work starts after a 4-row DMA).

Selection reproduces jnp.argmax's first-index tie-break exactly:
horizontal pass first (left/even column wins ties via is_ge), then vertical
(top row wins ties via a strict bottom-wins is_gt on the in-place select).
norm2 = fl(fl(re*re)+fl(im*im)) in f32 - selection is bit-exact with the
reference.  The selected payload is rounded to f16 (rel err ~1e-4 << the
2e-2 gate), halving output DMA traffic.

Engine/pipeline notes (measured on HW):
 - DVE is the bottleneck (~206us busy at 1.04 ns/elem/partition; the DVE
   2x/4x perf modes only engage for 2-byte tensor_scalar/tensor_copy
   forms that can't express the f32-exact compare chain, and GPSIMD
   offload stalls DVE ~2.5x via SBUF port contention, so all elastic work
   lives on ACT).
 - No V-select pre-fill: CP-V writes bottom-candidates onto the top slots
   of riH in place where the bottom strictly wins.  ACT compacts the
   strided winners into a contiguous f16 stage one subchunk later and
   issues the output DMA on the ACT HWDGE queue, so the ACT stream
   (square, prefH, compact, dma issue) never waits on DVE mid-stream and
   DVE runs gap-free.  The last subchunk stores straight from riH on the
   SP queue (strided 224B runs) to cut the drain tail.
 - Interleaving re/im pairs to shrink the copy_predicated ops was tried
   and is a net loss: pair-strided ACT writes run at ~2.2 ns/elem and
   their SBUF traffic slows concurrent DVE ops ~20%.
"""

import numpy as np

import concourse.bass as bass
import concourse.mybir as mybir
from concourse import bacc, bass_utils, tile

# Per-core shard geometry (hardcoded; kernel.py must be self-contained).
NCORES = 8
B = 2            # batch per core
RI = 2           # real/imag planes
C = 64           # channels
H = W = 224
HO, WO = H // 2, W // 2
P = 128          # SBUF partitions = B * C
R = 14           # image rows per regular compute subchunk

F32 = mybir.dt.float32
F16 = mybir.dt.float16
I8 = mybir.dt.uint8
OP = mybir.AluOpType
ACTF = mybir.ActivationFunctionType

# (row0, nrows) compute subchunks; first 14 rows split 4+10.  Measured
# best schedule: 6+8 is ~2us slower, and a finer 2+4+8 lead-in plus an
# 8+6 tail is also ~2us slower (the extra per-op overheads outweigh the
# smaller fill gap and drain).
SUBS = [(0, 4), (4, 10)] + [(14 * k, 14) for k in range(1, H // 14)]

_NC_CACHE = []


def _build_nc() -> bass.Bass:
    nc = bacc.Bacc("TRN2", target_bir_lowering=False, debug=False)
    # host pre-transposed: partition-major [b*c, ri, H, W] so every DMA is a
    # single-dim 128-partition transfer (hits all 16 SBUF AXI ports)
    x = nc.dram_tensor("x", [P, RI, H, W], F32, kind="ExternalInput").ap()
    out = nc.dram_tensor("out", [P, RI, HO, WO], F16, kind="ExternalOutput").ap()

    with tile.TileContext(nc) as tc:
        with tc.tile_pool(name="pool", bufs=2) as pool:
            xtiles = {}

            def emit_chunk_dma(sc):
                r0, nr = SUBS[sc]
                t = pool.tile([P, RI * R * W], F32, tag="xri", bufs=3,
                              name=f"xri{sc}")
                nc.sync.dma_start(
                    out=t[:, : RI * nr * W].rearrange(
                        "p (ri f) -> p ri f", ri=RI
                    ),
                    in_=x[:, :, r0 : r0 + nr, :].rearrange(
                        "p ri r w -> p ri (r w)"
                    ),
                )
                xtiles[sc] = t

            emit_chunk_dma(0)
            emit_chunk_dma(1)

            pend = None  # (riH5, out_row0, out_nrows) awaiting compact+store

            def emit_store():
                nonlocal pend
                if pend is None:
                    return
                riH5p, po0, pnr = pend
                stg = pool.tile([P, RI * (R // 2) * WO], F16, tag="stg",
                                name=f"stg{po0}")[:, : RI * pnr * WO]
                stg4 = stg.rearrange(
                    "p (ri rp w) -> p ri rp w", ri=RI, rp=pnr, w=WO
                )
                nc.scalar.copy(out=stg4, in_=riH5p[:, :, :, 0, :])
                nc.scalar.dma_start(
                    out=out[:, :, po0 : po0 + pnr, :].rearrange(
                        "p ri r w -> p ri (r w)"
                    ),
                    in_=stg.rearrange("p (ri f) -> p ri f", ri=RI),
                )
                pend = None

            for sc, (r0, nr) in enumerate(SUBS):
                t = xtiles[sc]
                n = nr * W                  # free elems per plane
                # prefetch: issue the next chunk's DMA as early as possible
                if sc + 2 < len(SUBS):
                    emit_chunk_dma(sc + 2)

                # subchunk views: [ri, r, w, t]
                xri6 = t[:, : RI * nr * W].rearrange(
                    "p (ri r w two) -> p ri r w two", ri=RI, r=nr, w=WO, two=2
                )

                # squares of re+im rows in one ACT op; norm2 in place over
                # the re half
                sqri = pool.tile([P, RI * R * W], F32, tag="sqri", bufs=3,
                                 name=f"sqri{sc}")[:, : RI * n]
                nc.scalar.activation(
                    out=sqri.rearrange(
                        "p (ri r w two) -> p ri r w two", ri=RI, r=nr, w=WO, two=2
                    ),
                    in_=xri6,
                    func=ACTF.Square,
                )

                # horizontal select pre-fill with the odd/right candidate
                # (ACT, casts f32->f16); depends only on the DMA + buffers
                riH = pool.tile([P, RI * R * WO], F16, tag="riH", bufs=3,
                                name=f"riH{sc}")[:, : RI * nr * WO]
                riH4 = riH.rearrange(
                    "p (ri r w) -> p ri r w", ri=RI, r=nr, w=WO
                )
                nc.scalar.copy(out=riH4, in_=xri6[:, :, :, :, 1])

                # compact+store the PREVIOUS subchunk's winners (its CP-V
                # finished while the squares above ran, so the ACT stream
                # still never blocks on DVE)
                emit_store()

                nrm = sqri[:, :n]
                nc.vector.tensor_tensor(
                    out=nrm, in0=nrm, in1=sqri[:, n:], op=OP.add
                )

                nrm4 = nrm.rearrange(
                    "p (r w two) -> p r w two", r=nr, w=WO, two=2
                )
                nE, nO = nrm4[:, :, :, 0], nrm4[:, :, :, 1]

                # horizontal mask (contiguous u8): even/left wins ties
                cH = pool.tile([P, R * WO], I8, tag="cH", bufs=3,
                               name=f"cH{sc}")[:, : nr * WO]
                cH3 = cH.rearrange("p (r w) -> p r w", r=nr, w=WO)
                nc.vector.tensor_tensor(out=cH3, in0=nE, in1=nO, op=OP.is_ge)
                # horizontal norm max -> nrm odd slots (in place)
                nc.vector.tensor_tensor(out=nO, in0=nE, in1=nO, op=OP.max)

                # horizontal select: overwrite pre-filled riH with the
                # even/left candidate where it wins
                cHb = cH3.unsqueeze(1).broadcast_to([P, RI, nr, WO])
                nc.vector.copy_predicated(
                    out=riH4, mask=cHb, data=xri6[:, :, :, :, 0]
                )

                # vertical mask from the horizontal maxes: bottom strictly
                # wins (top wins ties, matching argmax first-index)
                nrm5 = nrm.rearrange(
                    "p (rp rt w two) -> p rp rt w two",
                    rp=nr // 2, rt=2, w=WO, two=2,
                )
                cV = pool.tile([P, (R // 2) * WO], I8, tag="cV", bufs=3,
                               name=f"cV{sc}")[:, : (nr // 2) * WO]
                cV3 = cV.rearrange("p (rp w) -> p rp w", rp=nr // 2, w=WO)
                nc.vector.tensor_tensor(
                    out=cV3,
                    in0=nrm5[:, :, 1, :, 1],
                    in1=nrm5[:, :, 0, :, 1],
                    op=OP.is_gt,
                )

                # vertical select in place: bottom row onto the top slot
                # where the bottom strictly wins
                riH5 = riH.rearrange(
                    "p (ri rp rt w) -> p ri rp rt w",
                    ri=RI, rp=nr // 2, rt=2, w=WO,
                )
                cVb = cV3.unsqueeze(1).broadcast_to([P, RI, nr // 2, WO])
                nc.vector.copy_predicated(
                    out=riH5[:, :, :, 0, :], mask=cVb, data=riH5[:, :, :, 1, :]
                )

                if sc + 1 < len(SUBS):
                    pend = (riH5, r0 // 2, nr // 2)
                else:
                    # last subchunk: skip the ACT compact and store the
                    # winners strided straight from riH (SP queue) so the
                    # drain tail is just the transfer
                    nc.sync.dma_start(
                        out=out[:, :, r0 // 2 : r0 // 2 + nr // 2, :],
                        in_=riH5[:, :, :, 0, :],
                    )
    nc.compile()
    return nc


def get_nc() -> bass.Bass:
    if not _NC_CACHE:
        _NC_CACHE.append(_build_nc())
    return _NC_CACHE[0]


def kernel(x: np.ndarray, **run_kwargs) -> np.ndarray:
    nc = get_nc()
    xs = np.asarray(x, dtype=np.float32)
    assert xs.shape == (NCORES * B, RI, C, H, W), xs.shape
    # [16,2,64,H,W] -> per core [b,c,ri,H,W] flattened to [128,ri,H,W]
    xt = np.ascontiguousarray(xs.transpose(0, 2, 1, 3, 4))
    in_maps = [
        {"x": xt[B * i : B * (i + 1)].reshape(P, RI, H, W)} for i in range(NCORES)
    ]
    res = bass_utils.run_bass_kernel_spmd(
        nc, in_maps, core_ids=list(range(NCORES)), **run_kwargs
    )
    # per-core [128,ri,HO,WO] f16 -> [b,c,ri,HO,WO] -> [b,ri,c,HO,WO]
    out = np.concatenate(
        [
            np.asarray(res.results[i]["out"])
            .reshape(B, C, RI, HO, WO)
            .transpose(0, 2, 1, 3, 4)
            for i in range(NCORES)
        ],
        axis=0,
    )
    if run_kwargs:
        kernel.last_results = res
    return np.ascontiguousarray(out.astype(np.float32))



# revision 2
# speedup vs baseline: 1.1789x; 1.1789x over previous
"""Complex-magnitude MaxPool2d (k=2, s=2) Trainium2 Bass kernel.

Input  x:  [16, 2, 64, 224, 224] f32  (plane 0 = real, plane 1 = imag)
Output:    [16, 2, 64, 112, 112] f32  (value of the window element with the
                                       largest |z|^2 = re^2 + im^2)

Sharding: pure data parallel over batch: 16 / 8 cores = 2 examples per core.
Per core the 2(batch) x 64(channel) = 128 image planes map 1:1 onto the 128
SBUF partitions; compute runs on 14-row subchunks (the first 14 rows are
split 4+10 so the first DVE work starts after a 4-row DMA).

Selection reproduces jnp.argmax's first-index tie-break exactly:
horizontal pass first (left/even column wins ties), then vertical (top row
wins ties via a strict bottom-wins is_gt on the in-place select).  The
selected payload is rounded to f16 (rel err ~1e-4 << the 2e-2 gate),
halving output DMA traffic.

Engine/pipeline notes:
 - DVE is the bottleneck (1.04 ns/elem/partition f32; perf modes don't apply
   to this op mix — copy_predicated has none, tensor_tensor 2x needs all-f16
   packed operands which the f32 compare chain can't give).
 - CNORM_PAIRMAX: hand-written custom DVE op (per-NEFF uop table, 3-uop FSM
   alternating per element) streams re+im through two input ports and emits
   [n_even | pairmax] packed pairs in ONE 1x pass: out[2k] = re^2+im^2 at
   2k, out[2k+1] = max(n_2k, n_2k+1).  This replaces ACT Square (448/row),
   DVE add (224/row) and DVE horizontal max (112/row) with one 224/row DVE
   pass: DVE work 840 -> 728 elems/row/partition, ACT 784 -> 336.
   Norm arithmetic is f32 round-to-nearest, bit-identical to the
   ACT-square + DVE-add chain, so selection still matches argmax exactly.
 - Masks read the packed pairs: cH = is_ge(n_e, hmax) (== n_e >= n_o), and
   cV = is_gt on the hmax slots — same layout the baseline had after its
   in-place max.
 - ACT stream (prefill odd candidates, compact winners, output DMA issue on
   the ACT HWDGE queue) never waits on DVE mid-stream; last subchunk stores
   strided straight from riH on the SP queue to cut the drain tail.
 - GPSIMD offload stalls DVE ~2.5x via SBUF port contention — keep it idle.
"""

import numpy as np

import concourse.bass as bass
import concourse.mybir as mybir
import concourse.dve_ops as dve_ops
from concourse import bacc, bass_utils, tile
from concourse.dve_spec import Spec, Src0, Src1, Bin
from concourse.dve_uop import (
    ENABLE,
    AluInp,
    AluOp,
    DelayInp,
    DveOpSpec,
    InpSel,
    OutPath,
    OutSel,
    Trigger,
    UopConfig,
)

# Per-core shard geometry (hardcoded; kernel.py must be self-contained).
NCORES = 8
B = 2            # batch per core
RI = 2           # real/imag planes
C = 64           # channels
H = W = 224
HO, WO = H // 2, W // 2
P = 128          # SBUF partitions = B * C
R = 14           # image rows per regular compute subchunk

F32 = mybir.dt.float32
F16 = mybir.dt.float16
I8 = mybir.dt.uint8
OP = mybir.AluOpType

# (row0, nrows) compute subchunks; first 14 rows split 4+10 (measured best).
SUBS = [(0, 4), (4, 10)] + [(14 * k, 14) for k in range(1, H // 14)]

_NC_CACHE = []

# --- CNORM_PAIRMAX: hand-written custom DVE op ----------------------------- #
# Streams re (Src0) and im (Src1); per element computes n = re^2 + im^2.
# Even elements: park n in block 3's out-flop (BYPASS) and emit n.
# Odd elements: block 3 reads the parked value via CURR_ALU_OUT (same-block
# flop = previous element's value, one cycle stale) and emits MAX(n_o, n_e).
# FSM: evenA -> odd -> evenB -> odd -> ... via Trigger.COUNT, repeat_count=1;
# SRC_TENSOR_DONE (priority slot 0) idles at stream end.

_CNORM_NAME = "CNORM_PAIRMAX_ANT"


def _cnorm_phase(is_odd: bool) -> UopConfig:
    u = UopConfig()
    u.enable_input(InpSel.SRC_0, 1)  # lane0 <- re
    u.enable_input(InpSel.SRC_1, 2)  # lane1 <- im
    dp = u.datapath_config
    for st in range(8):
        dp[st].pass_through_delay(0, 1, 2)
    dp[0].enable_alu(AluOp.MULTIPLY, AluInp.PREV_DELAY_0, AluInp.PREV_DELAY_0)
    dp[1].enable_alu(AluOp.MULTIPLY, AluInp.PREV_DELAY_1, AluInp.PREV_DELAY_1)
    dp[1].enable_delay_from_src(DelayInp.PREV_ALU_OUT, 2)  # lane2 <- re^2
    dp[2].enable_alu(AluOp.ADD, AluInp.PREV_ALU_OUT, AluInp.PREV_DELAY_2)
    if is_odd:
        dp[3].enable_alu(AluOp.MAX, AluInp.PREV_ALU_OUT, AluInp.CURR_ALU_OUT)
    else:
        dp[3].enable_alu(AluOp.BYPASS, AluInp.PREV_ALU_OUT)
    for st in range(4, 8):
        dp[st].enable_alu(AluOp.BYPASS, AluInp.PREV_ALU_OUT)
    u.enable_output(OutSel.ALU_OUT, OutPath.WR0_LO)
    u.require_inp0 = ENABLE
    u.require_inp1 = ENABLE
    u.repeat_count = 1
    u.trigger = (Trigger.SRC_TENSOR_DONE, Trigger.COUNT, Trigger.NONE)
    return u


class _CnormPairmaxOp:
    """Quacks like dve_ops.DveOp; compile() returns hand-built uops."""

    name = _CNORM_NAME
    # Metadata decoy (leaves {Src0, Src1}, accum None); real semantics come
    # from the hand-built uop table below.
    spec = Spec(body=Bin(AluOp.ADD, Src0, Src1))
    subdim = False

    def __init__(self):
        self._compiled = {}

    def compile(self, ver) -> DveOpSpec:
        if ver not in self._compiled:
            even_a = _cnorm_phase(False)
            even_a.next_uop = (0, 1, 0)
            odd = _cnorm_phase(True)
            odd.next_uop = (0, 2, 0)
            even_b = _cnorm_phase(False)
            even_b.next_uop = (0, 1, 0)
            s = DveOpSpec(
                name=_CNORM_NAME,
                opcode=dve_ops.get_dve_sub_opcode(_CNORM_NAME),
                uops=[even_a, odd, even_b],
                rd1_en=True,
            )
            s.validate(ver)
            self._compiled[ver] = s
        return self._compiled[ver]


def _register_cnorm():
    for op in dve_ops.OPS:
        if op.name == _CNORM_NAME:
            return op
    op = _CnormPairmaxOp()
    dve_ops._SUB_OPCODE_FOR_NAME[_CNORM_NAME] = dve_ops._CUSTOM_DVE_ROW_BASE + len(
        dve_ops.OPS
    )
    dve_ops.OPS.append(op)
    dve_ops.CUSTOM_DVE_SPECS[_CNORM_NAME] = op.spec
    return op


# --------------------------------------------------------------------------- #


def _build_nc() -> bass.Bass:
    cnorm = _register_cnorm()
    nc = bacc.Bacc("TRN2", target_bir_lowering=False, debug=False)
    # host pre-transposed: partition-major [b*c, ri, H, W] so every DMA is a
    # single-dim 128-partition transfer (hits all 16 SBUF AXI ports)
    x = nc.dram_tensor("x", [P, RI, H, W], F32, kind="ExternalInput").ap()
    out = nc.dram_tensor("out", [P, RI, HO, WO], F16, kind="ExternalOutput").ap()

    with tile.TileContext(nc) as tc:
        with tc.tile_pool(name="pool", bufs=2) as pool:
            xtiles = {}

            def emit_chunk_dma(sc):
                r0, nr = SUBS[sc]
                t = pool.tile([P, RI * R * W], F32, tag="xri", bufs=3,
                              name=f"xri{sc}")
                nc.sync.dma_start(
                    out=t[:, : RI * nr * W].rearrange(
                        "p (ri f) -> p ri f", ri=RI
                    ),
                    in_=x[:, :, r0 : r0 + nr, :].rearrange(
                        "p ri r w -> p ri (r w)"
                    ),
                )
                xtiles[sc] = t

            emit_chunk_dma(0)
            emit_chunk_dma(1)

            pend = None  # (riH5, out_row0, out_nrows) awaiting compact+store

            def emit_store():
                nonlocal pend
                if pend is None:
                    return
                riH5p, po0, pnr = pend
                stg = pool.tile([P, RI * (R // 2) * WO], F16, tag="stg",
                                name=f"stg{po0}")[:, : RI * pnr * WO]
                stg4 = stg.rearrange(
                    "p (ri rp w) -> p ri rp w", ri=RI, rp=pnr, w=WO
                )
                nc.scalar.copy(out=stg4, in_=riH5p[:, :, :, 0, :])
                nc.scalar.dma_start(
                    out=out[:, :, po0 : po0 + pnr, :].rearrange(
                        "p ri r w -> p ri (r w)"
                    ),
                    in_=stg.rearrange("p (ri f) -> p ri f", ri=RI),
                )
                pend = None

            for sc, (r0, nr) in enumerate(SUBS):
                t = xtiles[sc]
                n = nr * W                  # free elems per plane
                # prefetch: issue the next chunk's DMA as early as possible
                if sc + 2 < len(SUBS):
                    emit_chunk_dma(sc + 2)

                # subchunk views: [ri, r, w, t]
                xri6 = t[:, : RI * nr * W].rearrange(
                    "p (ri r w two) -> p ri r w two", ri=RI, r=nr, w=WO, two=2
                )

                # horizontal select pre-fill with the odd/right candidate
                # (ACT, casts f32->f16); depends only on the DMA + buffers
                riH = pool.tile([P, RI * R * WO], F16, tag="riH", bufs=3,
                                name=f"riH{sc}")[:, : RI * nr * WO]
                riH4 = riH.rearrange(
                    "p (ri r w) -> p ri r w", ri=RI, r=nr, w=WO
                )
                nc.scalar.copy(out=riH4, in_=xri6[:, :, :, :, 1])

                # compact+store the PREVIOUS subchunk's winners on ACT
                emit_store()

                # norms + horizontal pair-max in ONE custom DVE pass:
                # nrm pairs = [n_even | max(n_even, n_odd)]
                nrm = pool.tile([P, R * W], F32, tag="nrm", bufs=3,
                                name=f"nrm{sc}")[:, :n]
                nc.vector._custom_dve(
                    cnorm, out=nrm, in0=t[:, :n], in1=t[:, n : 2 * n]
                )

                nrm4 = nrm.rearrange(
                    "p (r w two) -> p r w two", r=nr, w=WO, two=2
                )

                # horizontal mask: n_e >= hmax  <=>  n_e >= n_o
                # (even/left wins ties, matching argmax first-index)
                cH = pool.tile([P, R * WO], I8, tag="cH", bufs=3,
                               name=f"cH{sc}")[:, : nr * WO]
                cH3 = cH.rearrange("p (r w) -> p r w", r=nr, w=WO)
                nc.vector.tensor_tensor(
                    out=cH3, in0=nrm4[:, :, :, 0], in1=nrm4[:, :, :, 1],
                    op=OP.is_ge,
                )

                # vertical mask from the horizontal maxes: bottom strictly
                # wins (top wins ties, matching argmax first-index)
                nrm5 = nrm.rearrange(
                    "p (rp rt w two) -> p rp rt w two",
                    rp=nr // 2, rt=2, w=WO, two=2,
                )
                cV = pool.tile([P, (R // 2) * WO], I8, tag="cV", bufs=3,
                               name=f"cV{sc}")[:, : (nr // 2) * WO]
                cV3 = cV.rearrange("p (rp w) -> p rp w", rp=nr // 2, w=WO)
                nc.vector.tensor_tensor(
                    out=cV3,
                    in0=nrm5[:, :, 1, :, 1],
                    in1=nrm5[:, :, 0, :, 1],
                    op=OP.is_gt,
                )

                # horizontal select: overwrite pre-filled riH with the
                # even/left candidate where it wins
                cHb = cH3.unsqueeze(1).broadcast_to([P, RI, nr, WO])
                nc.vector.copy_predicated(
                    out=riH4, mask=cHb, data=xri6[:, :, :, :, 0]
                )

                # vertical select in place: bottom row onto the top slot
                # where the bottom strictly wins
                riH5 = riH.rearrange(
                    "p (ri rp rt w) -> p ri rp rt w",
                    ri=RI, rp=nr // 2, rt=2, w=WO,
                )
                cVb = cV3.unsqueeze(1).broadcast_to([P, RI, nr // 2, WO])
                nc.vector.copy_predicated(
                    out=riH5[:, :, :, 0, :], mask=cVb, data=riH5[:, :, :, 1, :]
                )

                if sc + 1 < len(SUBS):
                    pend = (riH5, r0 // 2, nr // 2)
                else:
                    # last subchunk: skip the ACT compact and store the
                    # winners strided straight from riH (SP queue) so the
                    # drain tail is just the transfer
                    nc.sync.dma_start(
                        out=out[:, :, r0 // 2 : r0 // 2 + nr // 2, :],
                        in_=riH5[:, :, :, 0, :],
                    )
    nc.compile()
    return nc


def get_nc() -> bass.Bass:
    if not _NC_CACHE:
        _NC_CACHE.append(_build_nc())
    return _NC_CACHE[0]


def kernel(x: np.ndarray, **run_kwargs) -> np.ndarray:
    nc = get_nc()
    xs = np.asarray(x, dtype=np.float32)
    assert xs.shape == (NCORES * B, RI, C, H, W), xs.shape
    # [16,2,64,H,W] -> per core [b,c,ri,H,W] flattened to [128,ri,H,W]
    xt = np.ascontiguousarray(xs.transpose(0, 2, 1, 3, 4))
    in_maps = [
        {"x": xt[B * i : B * (i + 1)].reshape(P, RI, H, W)} for i in range(NCORES)
    ]
    res = bass_utils.run_bass_kernel_spmd(
        nc, in_maps, core_ids=list(range(NCORES)), **run_kwargs
    )
    # per-core [128,ri,HO,WO] f16 -> [b,c,ri,HO,WO] -> [b,ri,c,HO,WO]
    out = np.concatenate(
        [
            np.asarray(res.results[i]["out"])
            .reshape(B, C, RI, HO, WO)
            .transpose(0, 2, 1, 3, 4)
            for i in range(NCORES)
        ],
        axis=0,
    )
    if run_kwargs:
        kernel.last_results = res
    return np.ascontiguousarray(out.astype(np.float32))
